# revision 5
# baseline (speedup 1.0000x reference)
"""Trainium2 Bass kernel for nn_CLFBlock (linear -> LIF scan -> linear -> T-mean -> log_softmax).

Self-contained: hardcodes shapes T=32, B=512, D=1024, C=1000 and data-parallel
sharding of the batch dim across 8 NeuronCores.

v3 schedule (ops identical to the proven baseline mix, pipeline rebuilt):
  - Host lays W1T out in j-major 128KB blocks and b1h/b2 pre-packed, so the
    DMA rings deliver exactly what the first matmul group needs first; the
    first mm1 MM issues ~1.5us after DMA flow starts instead of ~7us.
  - Junk matmuls (no data deps) run during the DMA fill to flip the PE HAM
    clock gate to 2.4GHz before real work arrives.
  - mm1 groups of tcnt=[4,4,8,8,8] timesteps: per-(g,j) PSUM tiles (1 bank,
    4-deep rotation), evicted to h_sb by Scalar ACT (scale 0.5/PRESCALE +
    0.5*b1 bias) right after each j's 4 DoubleRow MMs -> h streams into SBUF
    at 0.87us/t while the DVE scan consumes at ~1.03us/t, so the chain never
    starves and mm1 stays at the fp8-DoubleRow roofline.
  - LIF scan: fused custom DVE op (state update) + is_lt mask per step; spike
    sums accumulate on the PE (ident @ mask), deferred a few steps so mask
    waits never sit ahead of mm1 in the PE FIFO; junk MMs keep HAM warm
    through the scan tail so mm2 runs at full clock.
  - Epilogue: Exp table pre-warmed while the scan drains; y copies on the
    (idle) Vector engine; log-softmax; output DMA split in halves.
"""

import numpy as np
from contextlib import ExitStack

import concourse.bass as bass
import concourse.tile as tile
from concourse import bacc, mybir
from concourse.bass_utils import run_bass_kernel_spmd

N_CORES = 8
T, B, D, C = 32, 512, 1024, 1000
BC = B // N_CORES          # 64 rows per core
TB = T * BC                # 2048 matmul rows per core
FP32 = mybir.dt.float32
BF16 = mybir.dt.bfloat16
FP8 = mybir.dt.float8e4
W1_PRESCALE = 256.0   # host multiplies W1/W2 by this (exact power of 2) so
                      # their small uniform(-1/32,1/32) values stay in
                      # fp8e4m3's normal range; compensated on-chip
AF = mybir.ActivationFunctionType
OP = mybir.AluOpType

TCNTS = [4, 6, 6, 6, 6, 4]
OFFS = [0, 4, 10, 16, 22, 28, 32]


def _lif_op():
    """Fused LIF step: out = select(in0 < s0, in0, 0)*s1 + in1."""
    from concourse import dve_ops
    from concourse.dve_spec import Spec, Src0, Src1, Zero, C0, C1, select, lower
    from concourse.dve_uop import DveOpSpec

    for op in dve_ops.OPS:
        if op.name == "LIF_STEP_ANT":
            return op
    spec = Spec(
        body=select(Src0 < C0, Src0, Zero) * C1 + Src1,
        reference=lambda in0, in1, s0, s1, imm2: (
            np.where(in0.astype(np.float32) < s0, in0.astype(np.float32), 0.0) * s1
            + in1.astype(np.float32)).astype(np.float32),
    )
    row = dve_ops._CUSTOM_DVE_ROW_BASE + len(dve_ops.OPS)
    shas = {}
    for ver in ("v3", "v4"):
        try:
            shas[ver] = DveOpSpec(name="LIF_STEP_ANT", opcode=row,
                                  uops=lower(spec, ver=ver), rd1_en=True).sha(ver)
        except Exception:
            pass
    op = dve_ops.DveOp("LIF_STEP_ANT", spec, subdim=False, uops_sha=shas)
    dve_ops.OPS.append(op)
    dve_ops._SUB_OPCODE_FOR_NAME[op.name] = row
    dve_ops.CUSTOM_DVE_SPECS[op.name] = spec
    return op


def build_program():
    nc = bacc.Bacc("TRN2", target_bir_lowering=False, debug=False, num_devices=N_CORES)

    # W1T in j-major blocks: host layout [j=8][p=128][dj=8][e=128] so each
    # block is a contiguous 128KB transfer with 1KB rows
    w1j_d = nc.dram_tensor("W1J", [8, 128, 8 * 128], FP8, kind="ExternalInput").ap()
    xt_d = nc.dram_tensor("xT", [D, TB], FP8, kind="ExternalInput").ap()
    w2t_d = nc.dram_tensor("W2T", [D, C], FP8, kind="ExternalInput").ap()
    b1h_d = nc.dram_tensor("b1h", [128, 8], FP32, kind="ExternalInput").ap()
    b2_d = nc.dram_tensor("b2", [C], FP32, kind="ExternalInput").ap()
    y_d = nc.dram_tensor("y", [BC, C], FP32, kind="ExternalOutput").ap()

    lif = _lif_op()

    with tile.TileContext(nc) as tc, ExitStack() as ctx:
        sb = ctx.enter_context(tc.tile_pool(name="sb", bufs=1))
        mpool = ctx.enter_context(tc.tile_pool(name="mpool", bufs=10))

        # ---- constants with no DMA deps (warmup operands, scan state) ----
        io = sb.tile([128, 128], mybir.dt.int32)
        nc.gpsimd.iota(io[:], pattern=[[1, 128]], base=0, channel_multiplier=-1)
        ident = sb.tile([128, 128], BF16)
        nc.vector.tensor_scalar(ident[:], io[:], 0, None, op0=OP.is_equal)
        halfident = sb.tile([128, 128], BF16)
        nc.vector.tensor_scalar(halfident[:], io[:], 0, 0.5, op0=OP.is_equal,
                                op1=OP.mult)
        junkd = sb.tile([128, 512], BF16)
        nc.vector.memset(junkd[:], 0.125)
        wroll = sb.tile([128, 4 * 512], BF16)
        nc.vector.memset(wroll[:, 3 * 512:4 * 512], 0.0)   # w_{-1} = 0 in buf 3

        # ---- input DMA (two HWDGE rings, FIFO priority order):
        # b1h first (tiny, needed by the first eviction), then x cols 0-255,
        # then the 8 W1 j-blocks, then the rest of x, W2, b2 ----
        # w1t4: [p, j, dj, e]; one contiguous half-of-W1 transfer per ring
        w1t = sb.tile([128, 8 * 1024], FP8)
        w1t4 = w1t[:].rearrange("p (j k e) -> p j k e", j=8, k=8)
        w1thirds = w1t[:].rearrange("p (j x) -> p j x", j=8)
        w1src = w1j_d[:].rearrange("j p x -> p j x")
        # sync ring issues ~2us earlier than scalar: give it 6 of 8 W1 blocks
        nc.sync.dma_start(w1thirds[:, 0:6, :], w1src[:, 0:6, :])
        nc.scalar.dma_start(w1thirds[:, 6:8, :], w1src[:, 6:8, :])

        b1h = sb.tile([128, 8], FP32)
        nc.scalar.dma_start(b1h[:], b1h_d)

        xt = sb.tile([128, 8 * TB], FP8)
        xt3 = xt[:].rearrange("p (j t) -> p j t", j=8)
        xsrc = xt_d[:].rearrange("(dj p) t -> p dj t", p=128)

        def load_x_cols(c0, c1):
            nc.sync.dma_start(xt3[:, 0:4, c0:c1], xsrc[:, 0:4, c0:c1])
            nc.scalar.dma_start(xt3[:, 4:8, c0:c1], xsrc[:, 4:8, c0:c1])

        # first x chunks split across both rings right after each ring's W1
        load_x_cols(0, 256)
        load_x_cols(256, 512)
        load_x_cols(512, 1024)
        load_x_cols(1024, 1536)
        load_x_cols(1536, 2048)

        w2t = sb.tile([128, 8 * 1024], FP8)
        w2t3 = w2t[:].rearrange("p (j c) -> p j c", j=8)
        w2src = w2t_d[:].rearrange("(ej p) c -> p ej c", p=128)
        nc.sync.dma_start(w2t3[:, 0:4, 0:C], w2src[:, 0:4, :])
        nc.scalar.dma_start(w2t3[:, 4:8, 0:C], w2src[:, 4:8, :])
        b2_sb = sb.tile([1, C], FP32)
        nc.sync.dma_start(b2_sb[:], b2_d.rearrange("(a c) -> a c", a=1))

        # ---- persistent SBUF state ----
        h_sb = sb.tile([128, T * 512], BF16)
        h3 = h_sb[:].rearrange("p (t x) -> p t x", x=512)
        ssum = sb.tile([128, 512], FP8)
        y_sb = sb.tile([BC, 1024], FP32)
        ez = sb.tile([BC, 1024], FP32)

        with tc.tile_pool(name="ps_h", bufs=6, space="PSUM") as ps_h, \
             tc.tile_pool(name="psm", bufs=1, space="PSUM") as psm:

            msum = psm.tile([128, 512], FP32)
            # HAM warm-up during the DMA fill: tiny N=16 matmuls keep the PE
            # "busy" for the activity monitor at ~3% of a full MM's power (a
            # dense warmup measurably trips the chip-wide P0 power throttle
            # and slows every engine ~17%); msum's first real MM clears
            for i in range(55):
                nc.tensor.matmul(msum[:, 0:16], ident[:], junkd[:, 0:16],
                                 start=True, stop=True)

            n_msum = [0]
            masks = {}

            def emit_msum(t):
                ent = masks.pop(t, None)
                if ent is None:
                    return
                m, kind = ent
                if kind == "single":
                    nc.tensor.matmul(msum[:], ident[:], m[:],
                                     start=(n_msum[0] == 0),
                                     stop=(n_msum[0] == T - 1))
                    n_msum[0] += 1
                else:
                    # sign-pair: msum += 0.5*sign over both halves; the
                    # (1+sign)/2 offset is folded into the ssum bias
                    for half in range(2):
                        nc.tensor.matmul(msum[:], halfident[:],
                                         m[:, half * 512:(half + 1) * 512],
                                         start=(n_msum[0] == 0),
                                         stop=(n_msum[0] == T - 1))
                        n_msum[0] += 1

            def emit_group(g):
                t0, tcnt = OFFS[g], TCNTS[g]
                n = tcnt * 64
                for j in range(8):
                    ps = ps_h.tile([128, 512], FP32, tag="ps_h", name=f"ps_{g}_{j}")
                    for dp in range(4):
                        nc.tensor.matmul(
                            ps[:, 0:n],
                            w1t4[:, j, 2 * dp:2 * dp + 2, :],
                            xt3[:, 2 * dp:2 * dp + 2, t0 * 64:(t0 + tcnt) * 64],
                            start=(dp == 0), stop=(dp == 3),
                            perf_mode=mybir.MatmulPerfMode.DoubleRow,
                        )
                    nc.scalar.activation(
                        h3[:, t0:t0 + tcnt, j * 64:(j + 1) * 64],
                        ps[:, 0:n].rearrange("p (t b) -> p t b", t=tcnt),
                        AF.Identity, scale=0.5 / W1_PRESCALE, bias=b1h[:, j:j + 1],
                    )

            def emit_scan(t0, t1):
                for t in range(t0, t1):
                    a = wroll[:, ((t - 1) % 4) * 512:((t - 1) % 4) * 512 + 512]
                    b_ = wroll[:, (t % 4) * 512:(t % 4) * 512 + 512]
                    nc.vector._custom_dve(lif, out=b_, in0=a,
                                          in1=h_sb[:, t * 512:(t + 1) * 512],
                                          s0=1.0, s1=0.5)
                    if t >= T - 4:
                        # Scalar is idle once the last eviction is done: mask
                        # the final steps there as sign(1-w) over buffer pairs
                        if t % 2 == 1:
                            lo = ((t - 1) % 4) * 512
                            m = mpool.tile([128, 1024], BF16, tag="mp",
                                           name=f"mp{t}")
                            nc.scalar.activation(m[:], wroll[:, lo:lo + 1024],
                                                 AF.Sign, scale=-1.0, bias=1.0)
                            masks[t] = (m, "pair")
                    else:
                        m = mpool.tile([128, 512], BF16, tag="m", name=f"m{t}")
                        nc.vector.tensor_scalar(m[:], b_, 1.0, None, op0=OP.is_lt)
                        masks[t] = (m, "single")

            next_msum = [0]

            def flush_msum(up_to):
                while next_msum[0] < up_to:
                    emit_msum(next_msum[0])
                    next_msum[0] += 1

            for g in range(len(TCNTS)):
                if g >= 2:
                    flush_msum(OFFS[g] - 5)
                emit_group(g)
                emit_scan(OFFS[g], OFFS[g + 1])
            # scan tail: alternate spike-sum MMs with junk MMs (into a retired
            # ps_h buffer) to keep the PE HAM clock warm into mm2
            jk = ps_h.tile([128, 512], FP32, tag="ps_h", name="ps_junk")
            while next_msum[0] < T:
                emit_msum(next_msum[0])
                next_msum[0] += 1
                nc.tensor.matmul(jk[:, 0:192], ident[:], junkd[:, 0:192],
                                 start=True, stop=True)
            # ssum = T - msum (spike counts are small ints, exact in fp8)
            nc.scalar.activation(ssum[:], msum[:], AF.Copy, scale=-1.0,
                                 bias=float(T) - 2.0)

        ssum3 = ssum[:].rearrange("p (j b) -> p j b", j=8)
        b2_32 = sb.tile([1, C], BF16)
        nc.scalar.activation(b2_32[:], b2_sb[:], AF.Copy,
                             scale=float(T) * W1_PRESCALE)
        ones = sb.tile([1, BC], BF16)
        nc.vector.memset(ones[:], 1.0)
        # warm the Exp spline table AFTER the Copy-family ACTs (Copy evicts
        # the spline slot); its load overlaps the mm2 matmuls
        nc.scalar.activation(ez[0:1, 1004:1008], b2_sb[0:1, 0:4], AF.Exp)

        with tc.tile_pool(name="psy", bufs=2, space="PSUM") as psy_pool:
            for half in range(2):
                n = 512 if half == 0 else C - 512
                c0 = half * 512
                psy = psy_pool.tile([BC, 512], FP32, tag="ps_y", name=f"psy{half}")
                for pj in range(4):
                    nc.tensor.matmul(
                        psy[:, 0:n],
                        ssum3[:, 2 * pj:2 * pj + 2, :],
                        w2t3[:, 2 * pj:2 * pj + 2, c0:c0 + n],
                        start=(pj == 0), stop=False,
                        perf_mode=mybir.MatmulPerfMode.DoubleRow,
                    )
                nc.tensor.matmul(psy[:, 0:n], ones[:], b2_32[:, c0:c0 + n],
                                 start=False, stop=True)
                nc.vector.tensor_scalar(y_sb[:, c0:c0 + n], psy[:, 0:n],
                                        1.0 / (float(T) * W1_PRESCALE), None,
                                        op0=OP.mult)

        # ---- log_softmax over C (|y| small enough that no max-shift needed);
        # exp/reduce pipelined in halves across Scalar/Vector ----
        red = sb.tile([BC, 2], FP32)
        nc.scalar.activation(ez[:, 0:512], y_sb[:, 0:512], AF.Exp)
        nc.vector.reduce_sum(red[:, 0:1], ez[:, 0:512], axis=mybir.AxisListType.X)
        nc.scalar.activation(ez[:, 512:C], y_sb[:, 512:C], AF.Exp)
        nc.scalar.activation(ez[0:1, 1000:1004], b2_sb[0:1, 0:4], AF.Ln)
        nc.vector.reduce_sum(red[:, 1:2], ez[:, 512:C], axis=mybir.AxisListType.X)
        ssum_e = sb.tile([BC, 1], FP32)
        nc.vector.tensor_tensor(ssum_e[:], red[:, 0:1], red[:, 1:2], op=OP.add)
        lse = sb.tile([BC, 1], FP32)
        nc.scalar.activation(lse[:], ssum_e[:], AF.Ln)
        out_sb = sb.tile([BC, C], FP32)
        nc.vector.tensor_scalar(out_sb[:, 0:512], y_sb[:, 0:512], lse[:], None,
                                op0=OP.subtract)
        nc.sync.dma_start(y_d[:, 0:512], out_sb[:, 0:512])
        nc.vector.tensor_scalar(out_sb[:, 512:C], y_sb[:, 512:C], lse[:], None,
                                op0=OP.subtract)
        nc.sync.dma_start(y_d[:, 512:C], out_sb[:, 512:C])

    nc.compile()
    return nc


_CACHE = {}


def kernel(x, W1, b1, W2, b2):
    if "nc" not in _CACHE:
        _CACHE["nc"] = build_program()
    nc = _CACHE["nc"]

    f8 = mybir.dt.np(FP8)
    x = np.asarray(x, dtype=np.float32)
    w1t = (np.asarray(W1, dtype=np.float32).T * W1_PRESCALE).astype(f8)
    # j-major blocks: [j, p, dj, e] from W1T [dj*128+p, j*128+e]
    w1j = np.ascontiguousarray(
        w1t.reshape(8, 128, 8, 128).transpose(2, 1, 0, 3)).reshape(8, 128, 8 * 128)
    w2t = np.ascontiguousarray(
        (np.asarray(W2, dtype=np.float32).T * W1_PRESCALE).astype(f8))
    b1h = np.ascontiguousarray(
        (0.5 * np.asarray(b1, dtype=np.float32)).reshape(8, 128).T)
    b2 = np.ascontiguousarray(b2, dtype=np.float32)
    in_maps = []
    for i in range(N_CORES):
        xs = np.ascontiguousarray(
            x[:, i * BC:(i + 1) * BC, :].reshape(TB, D).T.astype(f8))
        in_maps.append({"xT": xs, "W1J": w1j, "W2T": w2t, "b1h": b1h, "b2": b2})

    res = run_bass_kernel_spmd(nc, in_maps, core_ids=list(range(N_CORES)),
                               **_CACHE.get("run_kwargs", {}))
    _CACHE["last_results"] = res
    out = np.concatenate([res.results[i]["y"] for i in range(N_CORES)], axis=0)
    return out
